# revision 8
# baseline (speedup 1.0000x reference)
"""Bass TRN2 kernel for nn_EtaWeights.

out[i] = loss[i]*mask*eta   if loss[i] > eta
       = -loss[i]/eta + 1   otherwise

Data-parallel over the single axis: 8 cores, each streams a contiguous
2^22-element shard of the 2^25-element vector through SBUF.

Fast path (mask*eta == 0, the shipped parameter values): the true-branch is
identically 0 and the false-branch 1 - x/eta crosses zero exactly at x = eta,
so out == -min(x - eta, 0) / eta exactly (fp32 rounding is symmetric under
negation, and the +/-0 difference on the clamped branch is value-equal).
Pipeline: SP issues in-DMAs (HWDGE), DVE runs one fused tensor_scalar
(subtract, min) in place, ACT scales by -1/eta (Copy activation) and issues
the out-DMA in program order. Raw Bass with explicit slot semaphores — Tile
would attach >1 sync-wait to DMA instructions, which walrus rejects.

General path (mask*eta != 0): all-DVE compare + predicated copy; ACT only
issues out-DMAs.
"""

import numpy as np

N = 33554432  # 2**25
NCORES = 8
PER_CORE = N // NCORES  # 2**22

P = 128  # SBUF partitions
NT = 8  # tiles per core
F = PER_CORE // (NT * P)  # 4096 -> 2 MiB per tile
BUFS = 6

TRACE = False
LAST_EXEC_NS = None
LAST_RESULTS = None

_module_cache = {}


def _build(e: float, m: float, nt: int = NT, f: int = F, repeats: int = 1,
           bufs: int = BUFS):
    import concourse.bass as bass
    import concourse.mybir as mybir

    fp32 = mybir.dt.float32
    alu = mybir.AluOpType
    nc = bass.Bass("TRN2", target_bir_lowering=False, debug=False,
                   num_devices=NCORES)
    x = nc.dram_tensor("x", [nt, P, f], fp32, kind="ExternalInput").ap()
    y = nc.dram_tensor("y", [nt, P, f], fp32, kind="ExternalOutput").ap()

    total = nt * repeats
    fast = m * e == 0.0

    with (
        nc.sbuf_tensor([P, f * bufs], fp32) as buf,
        nc.sbuf_tensor([P, f * 2], fp32) as aux,
        nc.Block(no_gpsimd_drain=True) as block,
    ):
        tiles = [buf[:, k * f:(k + 1) * f] for k in range(bufs)]
        gt_t = aux[:, 0:f]
        tr_t = aux[:, f:2 * f]
        in_sems = [nc.alloc_semaphore(f"in{k}") for k in range(bufs)]
        out_sems = [nc.alloc_semaphore(f"out{k}") for k in range(bufs)]
        dve_sem = nc.alloc_semaphore("dve")
        act_sem = nc.alloc_semaphore("act")
        uses = [len(range(k, total, bufs)) for k in range(bufs)]

        @block.sync
        def _(sp):
            for it in range(total):
                k, u = it % bufs, it // bufs
                if u > 0:
                    sp.wait_ge(out_sems[k], 16 * u)
                sp.dma_start(tiles[k], x[it % nt]).then_inc(in_sems[k], 16)
            for k in range(bufs):
                sp.wait_ge(out_sems[k], 16 * uses[k])

        @block.vector
        def _(dve):
            for it in range(total):
                k, u = it % bufs, it // bufs
                dve.wait_ge(in_sems[k], 16 * (u + 1))
                if fast:
                    # t = min(x - e, 0); ACT then scales by -1/e
                    dve.tensor_scalar(
                        tiles[k], tiles[k], e, 0.0, alu.subtract, alu.min
                    ).then_inc(dve_sem, 1)
                else:
                    # fully serialized on DVE (deep pipeline needs explicit
                    # sems even for same-engine dependencies); ACT waits for
                    # 5 chain ticks per iteration
                    ops = [
                        lambda: dve.tensor_scalar(gt_t, tiles[k], e, None,
                                                  alu.is_gt),
                        lambda: dve.tensor_scalar(tr_t, tiles[k], m * e,
                                                  None, alu.mult),
                        lambda: dve.tensor_scalar(tiles[k], tiles[k], e, 0.0,
                                                  alu.subtract, alu.min),
                        lambda: dve.tensor_scalar(tiles[k], tiles[k],
                                                  -1.0 / e, None, alu.mult),
                        lambda: dve.copy_predicated(tiles[k], gt_t, tr_t),
                    ]
                    for j, op in enumerate(ops):
                        dve.wait_ge(dve_sem, 5 * it + j)
                        op().then_inc(dve_sem, 1)

        @block.scalar
        def _(act):
            for it in range(total):
                k = it % bufs
                act.wait_ge(dve_sem, (it + 1) if fast else 5 * (it + 1))
                if fast:
                    # deep ACT pipeline: the HWDGE DMA issued by ACT does not
                    # implicitly wait for ACT's own in-flight compute
                    act.mul(tiles[k], tiles[k], -1.0 / e).then_inc(act_sem, 1)
                    act.wait_ge(act_sem, it + 1)
                act.dma_start(y[it % nt], tiles[k]).then_inc(out_sems[k], 16)

    return nc


def kernel(loss: np.ndarray, eta: np.ndarray, mask: np.ndarray) -> np.ndarray:
    global LAST_EXEC_NS, LAST_RESULTS
    from concourse.bass_utils import run_bass_kernel_spmd

    loss = np.ascontiguousarray(np.asarray(loss, dtype=np.float32))
    e = float(np.asarray(eta).reshape(-1)[0])
    m = float(np.asarray(mask).reshape(-1)[0])
    assert loss.shape == (N,)

    key = (e, m)
    if key not in _module_cache:
        _module_cache[key] = _build(e, m)
    nc = _module_cache[key]

    shards = loss.reshape(NCORES, NT, P, F)
    in_maps = [{"x": shards[c]} for c in range(NCORES)]
    res = run_bass_kernel_spmd(
        nc, in_maps, core_ids=list(range(NCORES)), trace=TRACE
    )
    LAST_EXEC_NS = res.exec_time_ns
    LAST_RESULTS = res
    out = np.concatenate(
        [np.asarray(r["y"], dtype=np.float32).reshape(-1) for r in res.results]
    )
    return out
